# revision 6
# baseline (speedup 1.0000x reference)
"""Binarized BasicBlock (BNN) forward on 8 Trainium2 NeuronCores.

Reference computation (per reference.py):
    xb  = sign(x);  wb = sign(w)
    y1  = conv3x3(xb, wb1, pad=1)
    a1  = hardtanh(bn1(y1))          # only sign(a1) feeds conv2
    y2  = conv3x3(sign(a1), wb2, pad=1)
    out = hardtanh(bn2(y2) + x)

Strategy:
  - Data parallel: batch N=64 -> 8 images per core; weights/BN replicated.
  - Conv as 9 shifted matmuls over a zero-padded 58x58 image held in SBUF,
    contraction over input channels: 256 channels = 2 planes of 128
    partitions contracted in ONE matmul via fp8 DoubleRow perf mode.
  - Binarized operands stored as fp8e4 (+-1, 0 exact); PSUM accumulates
    fp32; sums of +-1 with <=2304 terms are exact integers in fp32.
  - BN folded into the activation op: sign(bn1(y)) = Sign(y*s1 + t1) with
    s1 = g1/sqrt(v1+eps), t1 = b1 - m1*s1 (host-folded, passed as inputs).
  - Final stage: Identity(y2*s2+t2) on ScalarE, then add-residual and
    clip (tensor_scalar min+max) on VectorE.
  - The PE array streams 1 output column/cycle (fp8 DoubleRow): hard floor
    ~381us/core incl per-MM NX issue overhead.  Head/tail tuning: input
    planes split across the sync/gpsimd DMA queues, BN packed into one
    transfer, a short small-FD warm-up bridges the preamble to the first
    real matmul, input n+1 is prefetched during image n, and output
    stores go on the sync(co0)/scalar(co1) queues *behind* the prefetch
    so no queue ever backs up in front of an input load.
"""

import sys

try:
    import concourse  # noqa: F401
except ImportError:  # pragma: no cover
    sys.path.insert(0, "/opt/trn_rl_repo")

import numpy as np
import ml_dtypes

import concourse.bacc as bacc
import concourse.tile as tile
import concourse.mybir as mybir
from concourse.bass_utils import run_bass_kernel_spmd

dt = mybir.dt
AF = mybir.ActivationFunctionType
ALU = mybir.AluOpType
PM = mybir.MatmulPerfMode

N_CORES = 8
NPER = 8          # images per core
C = 256
H = W = 56
HW = H * W        # 3136
WP = 58           # padded row width (1 + 56 + 1)
ROWW = 64         # allocated width per (row, k-plane) block (16B aligned)
RPITCH = 2 * ROWW  # 128 = row pitch (both k-planes interleaved per row)
PROWS = 58        # padded rows
PLSZ = PROWS * RPITCH  # 7424 = padded image tile length
RPC = 8           # output rows per matmul chunk
CHU = RPC * W     # 448 = useful matmul free dim (garbage cols skipped)
NCH = H // RPC    # 7 chunks per image
BN_EPS = 1e-5
N_WARM = 22       # small warm-up matmuls bridging preamble -> first real MM

_CACHE = {}


def _zero_pads(nc, t):
    """Zero the padding cells of a [128, PLSZ] row-interleaved image tile.

    Layout: element (row r, k-plane k, col c) at r*RPITCH + k*ROWW + c;
    c=1..56 hold image cols 0..55, c=0 and c=57..63 are zero pads, rows
    0 and 57 are zero pad rows."""
    v = t[:]
    nc.gpsimd.memset(v[:, 0:RPITCH], 0.0)                      # top pad row
    nc.gpsimd.memset(v[:, 57 * RPITCH:PLSZ], 0.0)              # bottom pad row
    # per-block right pads c=57..63 plus the following block's c=0
    cols = v[:, 57:57 + 57 * RPITCH].rearrange("p (r k c) -> p r k c", k=2, c=ROWW)
    nc.gpsimd.memset(cols[:, :, :, 0:8], 0.0)


def _rview(t):
    # [128, PROWS, 2, ROWW]
    return t[:].rearrange("p (r k c) -> p r k c", k=2, c=ROWW)


def _build():
    nc = bacc.Bacc("TRN2", target_bir_lowering=False, debug=False)

    x_d = nc.dram_tensor("x", [NPER, C, H, W], dt.float32, kind="ExternalInput").ap()
    # weights: [co_chunk 2, cin_chunk 2, cin 128, tap 9, co 128]
    w1_d = nc.dram_tensor("w1b", [2, 2, 128, 9, 128], dt.float8e4, kind="ExternalInput").ap()
    w2_d = nc.dram_tensor("w2b", [2, 2, 128, 9, 128], dt.float8e4, kind="ExternalInput").ap()
    # bn holds [s1, t1, s2, t2] x [co_chunk 2] x [128]
    bn_d = nc.dram_tensor("bn", [4, 2, 128], dt.float32, kind="ExternalInput").ap()
    out_d = nc.dram_tensor("out", [NPER, C, H, W], dt.float32, kind="ExternalOutput").ap()

    with tile.TileContext(nc) as tc:
        with (
            tc.tile_pool(name="wp", bufs=1) as wp,
            tc.tile_pool(name="xin", bufs=4) as xinp,
            tc.tile_pool(name="xb", bufs=2) as xbp,
            tc.tile_pool(name="ab", bufs=2) as abp,
            tc.tile_pool(name="ost", bufs=3) as ostp,
            tc.tile_pool(name="tmp", bufs=4) as tmpp,
            tc.tile_pool(name="ps", bufs=7, space="PSUM") as psp,
            nc.sbuf_tensor([128, 2, 128], dt.float8e4) as warm_in,
            nc.psum_tensor([128, 128], dt.float32) as warm_ps,
        ):
            w_sb = []
            for tag in ("w1", "w2"):
                w_sb.append(
                    wp.tile([128, 2, 2, 9, 128], dt.float8e4, tag=tag,
                            name=f"w_{tag}"))
            bn_sb = wp.tile([128, 4, 2], dt.float32, tag="bn")

            # ---- head: queue-balanced loads so the first chunk's data,
            # weights and BN params all land by ~8-9.5us.
            # scalar queue: BN (one packed transfer) + w1 co-half 1.
            nc.scalar.dma_start(bn_sb[:], bn_d.rearrange("f q p -> p f q"))
            nc.scalar.dma_start(
                w_sb[0][:, 1],
                w1_d[1].rearrange("q p k c -> p q k c"))

            # image 0, piecewise: q0 rows on sync, q1 rows on gpsimd; Sign
            # into the padded xb tile right after each piece lands.
            pieces0 = ((0, 12), (12, 16), (28, 28))
            xb_cur = xbp.tile([128, PLSZ], dt.float8e4, tag="xb")
            xbv0 = _rview(xb_cur)
            xin_cur = [
                xinp.tile([128, HW], dt.float32, tag="xin", name=f"xin0_{q}")
                for q in range(2)
            ]
            for r0, nr in pieces0:
                for q in range(2):
                    dma_eng = nc.sync if q == 0 else nc.gpsimd
                    dma_eng.dma_start(
                        xin_cur[q][:, r0 * W:(r0 + nr) * W],
                        x_d[0, q * 128:(q + 1) * 128, r0:r0 + nr].rearrange(
                            "p h w -> p (h w)"),
                    )
                    nc.scalar.activation(
                        xbv0[:, 1 + r0:1 + r0 + nr, q, 1:57],
                        xin_cur[q][:, r0 * W:(r0 + nr) * W].rearrange(
                            "p (h w) -> p h w", w=W),
                        AF.Sign,
                    )
            # w1 co-half 0 on sync behind image-0's q0 pieces (needed by the
            # first matmul at ~9.5us; ready ~8.5us); w2 on gpsimd behind the
            # q1 pieces and the xb pad memsets (needed only from conv2 on).
            nc.sync.dma_start(
                w_sb[0][:, 0],
                w1_d[0].rearrange("q p k c -> p q k c"))
            _zero_pads(nc, xb_cur)
            for co in range(2):
                nc.gpsimd.dma_start(
                    w_sb[1][:, co], w2_d[co].rearrange("q p k c -> p q k c"))

            # ---- PE warm-up: small junk matmuls (FD=64) so the PE never
            # idles between the framework preamble and the first real matmul
            # (an idle >3.4us re-throttles the HAM clock gate).
            nc.gpsimd.memset(warm_in[:], 0.0)
            for _ in range(N_WARM):
                nc.tensor.matmul(
                    warm_ps[:, 0:64], warm_in[:], warm_in[:, :, 0:64],
                    start=True, stop=True, perf_mode=PM.DoubleRow,
                )

            def prefetch_image(n):
                """DMA image n (q0 on sync, q1 on gpsimd) and binarize into a
                fresh padded xb tile.  Returns (xb_tile, [xin_q0, xin_q1])."""
                xb = xbp.tile([128, PLSZ], dt.float8e4, tag="xb")
                xbv = _rview(xb)
                xin = []
                for q in range(2):
                    xi = xinp.tile([128, HW], dt.float32, tag="xin")
                    xin.append(xi)
                    dma_eng = nc.sync if q == 0 else nc.gpsimd
                    dma_eng.dma_start(
                        xi[:],
                        x_d[n, q * 128:(q + 1) * 128].rearrange("p h w -> p (h w)"),
                    )
                    nc.scalar.activation(
                        xbv[:, 1:57, q, 1:57],
                        xi[:].rearrange("p (h w) -> p h w", w=W),
                        AF.Sign,
                    )
                _zero_pads(nc, xb)
                return xb, xin

            for n in range(NPER):
                xbv = _rview(xb_cur)

                # ---- conv1 -> sign(bn1(.)) into padded intermediate ----
                ab = abp.tile([128, PLSZ], dt.float8e4, tag="ab")
                _zero_pads(nc, ab)
                abv = _rview(ab)
                for co in range(2):
                    for s in range(NCH):
                        ps = psp.tile([128, CHU], dt.float32, tag="ps")
                        for kk in range(9):
                            r0 = RPC * s + kk // 3
                            rhs = xbv[:, r0:r0 + RPC, :, kk % 3:kk % 3 + W].rearrange(
                                "p r k c -> p k r c")
                            nc.tensor.matmul(
                                ps[:],
                                w_sb[0][:, co, :, kk, :],
                                rhs,
                                start=(kk == 0),
                                stop=(kk == 8),
                                perf_mode=PM.DoubleRow,
                            )
                        psv = ps[:].rearrange("p (r c) -> p r c", c=W)
                        nc.scalar.activation(
                            abv[:, 1 + RPC * s:1 + RPC * s + RPC, co, 1:57], psv, AF.Sign,
                            bias=bn_sb[:, 1, co:co + 1], scale=bn_sb[:, 0, co:co + 1],
                        )

                # ---- prefetch + binarize input of image n+1 ----
                # DMAs start now; the Sign ops sit between conv1's and conv2's
                # epilogue activations on the in-order ScalarE queue, done
                # long before conv1 of image n+1 needs them.
                if n + 1 < NPER:
                    xb_nxt, xin_nxt = prefetch_image(n + 1)

                # ---- conv2 -> bn2 + residual + clip ----
                for co in range(2):
                    ost = ostp.tile([128, HW], dt.float32, tag="ost")
                    ostv = ost[:].rearrange("p (h w) -> p h w", w=W)
                    xinv = xin_cur[co][:].rearrange("p (h w) -> p h w", w=W)
                    st_eng = nc.sync if co == 0 else nc.scalar
                    for s in range(NCH):
                        ps = psp.tile([128, CHU], dt.float32, tag="ps")
                        for kk in range(9):
                            r0 = RPC * s + kk // 3
                            rhs = abv[:, r0:r0 + RPC, :, kk % 3:kk % 3 + W].rearrange(
                                "p r k c -> p k r c")
                            nc.tensor.matmul(
                                ps[:],
                                w_sb[1][:, co, :, kk, :],
                                rhs,
                                start=(kk == 0),
                                stop=(kk == 8),
                                perf_mode=PM.DoubleRow,
                            )
                        psv = ps[:].rearrange("p (r c) -> p r c", c=W)
                        tm = tmpp.tile([128, RPC * W], dt.float32, tag="tmp")
                        tmv = tm[:].rearrange("p (r c) -> p r c", c=W)
                        nc.scalar.activation(
                            tmv, psv, AF.Identity,
                            bias=bn_sb[:, 3, co:co + 1], scale=bn_sb[:, 2, co:co + 1],
                        )
                        ov = ostv[:, RPC * s:RPC * s + RPC, :]
                        nc.vector.tensor_tensor(
                            ov, tmv, xinv[:, RPC * s:RPC * s + RPC, :], ALU.add
                        )
                        nc.vector.tensor_scalar(ov, ov, 1.0, -1.0, ALU.min, ALU.max)
                        if s == 3:
                            st_eng.dma_start(
                                out_d[n, co * 128:(co + 1) * 128, 0:32].rearrange(
                                    "p h w -> p (h w)"),
                                ost[:, 0:32 * W],
                            )
                        elif s >= 4:
                            r0o, r1o = 8 * s, 8 * s + 8
                            st_eng.dma_start(
                                out_d[n, co * 128:(co + 1) * 128, r0o:r1o].rearrange(
                                    "p h w -> p (h w)"),
                                ost[:, r0o * W:r1o * W],
                            )

                if n + 1 < NPER:
                    xb_cur, xin_cur = xb_nxt, xin_nxt

    nc.compile()
    return nc


def _get_nc():
    if "nc" not in _CACHE:
        _CACHE["nc"] = _build()
    return _CACHE["nc"]


def _prep_weights(w):
    # [co, cin, kh, kw] ->
    # [co_chunk 2, cin_chunk 2, cin 128, tap 9, co 128], binarized fp8e4
    a = np.sign(w.astype(np.float32))          # [co 256, ci 256, 3, 3]
    a = a.reshape(2, 128, 2, 128, 9)           # [coq, c, q, p, k]
    a = a.transpose(0, 2, 3, 4, 1)             # [coq, q, p, k, c]
    return np.ascontiguousarray(a.astype(ml_dtypes.float8_e4m3))


def _fold_bn(g, b, m, v):
    s = (g.astype(np.float32) / np.sqrt(v.astype(np.float32) + BN_EPS)).astype(np.float32)
    t = (b.astype(np.float32) - m.astype(np.float32) * s).astype(np.float32)
    return s.reshape(2, 128), t.reshape(2, 128)


def _make_in_maps(x, w1, g1, b1, m1, v1, w2, g2, b2, m2, v2):
    w1b = _prep_weights(w1)
    w2b = _prep_weights(w2)
    s1, t1 = _fold_bn(g1, b1, m1, v1)
    s2, t2 = _fold_bn(g2, b2, m2, v2)
    bn = np.ascontiguousarray(np.stack([s1, t1, s2, t2], axis=0))
    x = np.ascontiguousarray(x.astype(np.float32, copy=False))
    return [
        {
            "x": x[c * NPER:(c + 1) * NPER],
            "w1b": w1b, "w2b": w2b, "bn": bn,
        }
        for c in range(N_CORES)
    ]


def kernel(x, w1, g1, b1, m1, v1, w2, g2, b2, m2, v2):
    nc = _get_nc()
    in_maps = _make_in_maps(x, w1, g1, b1, m1, v1, w2, g2, b2, m2, v2)
    res = run_bass_kernel_spmd(nc, in_maps, list(range(N_CORES)))
    out = np.concatenate([res.results[c]["out"] for c in range(N_CORES)], axis=0)
    return out


# revision 7
# speedup vs baseline: 1.0011x; 1.0011x over previous
"""Binarized BasicBlock (BNN) forward on 8 Trainium2 NeuronCores.

Reference computation (per reference.py):
    xb  = sign(x);  wb = sign(w)
    y1  = conv3x3(xb, wb1, pad=1)
    a1  = hardtanh(bn1(y1))          # only sign(a1) feeds conv2
    y2  = conv3x3(sign(a1), wb2, pad=1)
    out = hardtanh(bn2(y2) + x)

Strategy:
  - Data parallel: batch N=64 -> 8 images per core; weights/BN replicated.
  - Conv as 9 shifted matmuls over a zero-padded 58x58 image held in SBUF,
    contraction over input channels: 256 channels = 2 planes of 128
    partitions contracted in ONE matmul via fp8 DoubleRow perf mode.
  - Binarized operands stored as fp8e4 (+-1, 0 exact); PSUM accumulates
    fp32; sums of +-1 with <=2304 terms are exact integers in fp32.
  - BN folded into the activation op: sign(bn1(y)) = Sign(y*s1 + t1) with
    s1 = g1/sqrt(v1+eps), t1 = b1 - m1*s1 (host-folded, passed as inputs).
  - Final stage: Identity(y2*s2+t2) on ScalarE, then add-residual and
    clip (tensor_scalar min+max) on VectorE.
  - The PE array streams 1 output column/cycle (fp8 DoubleRow): hard floor
    ~381us/core incl per-MM NX issue overhead.  Head/tail tuning: input
    planes split across the sync/gpsimd DMA queues, BN packed into one
    transfer, a short small-FD warm-up bridges the preamble to the first
    real matmul, input n+1 is prefetched during image n, and output
    stores go on the sync(co0)/scalar(co1) queues *behind* the prefetch
    so no queue ever backs up in front of an input load.
"""

import sys

try:
    import concourse  # noqa: F401
except ImportError:  # pragma: no cover
    sys.path.insert(0, "/opt/trn_rl_repo")

import numpy as np
import ml_dtypes

import concourse.bacc as bacc
import concourse.tile as tile
import concourse.mybir as mybir
from concourse.bass_utils import run_bass_kernel_spmd

dt = mybir.dt
AF = mybir.ActivationFunctionType
ALU = mybir.AluOpType
PM = mybir.MatmulPerfMode

N_CORES = 8
NPER = 8          # images per core
C = 256
H = W = 56
HW = H * W        # 3136
WP = 58           # padded row width (1 + 56 + 1)
ROWW = 64         # allocated width per (row, k-plane) block (16B aligned)
RPITCH = 2 * ROWW  # 128 = row pitch (both k-planes interleaved per row)
PROWS = 58        # padded rows
PLSZ = PROWS * RPITCH  # 7424 = padded image tile length
RPC = 8           # output rows per matmul chunk
CHU = RPC * W     # 448 = useful matmul free dim (garbage cols skipped)
NCH = H // RPC    # 7 chunks per image
BN_EPS = 1e-5
N_WARM = 32       # small warm-up matmuls bridging preamble -> first real MM

_CACHE = {}


def _zero_pads(nc, t):
    """Zero the padding cells of a [128, PLSZ] row-interleaved image tile.

    Layout: element (row r, k-plane k, col c) at r*RPITCH + k*ROWW + c;
    c=1..56 hold image cols 0..55, c=0 and c=57..63 are zero pads, rows
    0 and 57 are zero pad rows."""
    v = t[:]
    nc.gpsimd.memset(v[:, 0:RPITCH], 0.0)                      # top pad row
    nc.gpsimd.memset(v[:, 57 * RPITCH:PLSZ], 0.0)              # bottom pad row
    # per-block right pads c=57..63 plus the following block's c=0
    cols = v[:, 57:57 + 57 * RPITCH].rearrange("p (r k c) -> p r k c", k=2, c=ROWW)
    nc.gpsimd.memset(cols[:, :, :, 0:8], 0.0)


def _rview(t):
    # [128, PROWS, 2, ROWW]
    return t[:].rearrange("p (r k c) -> p r k c", k=2, c=ROWW)


def _build():
    nc = bacc.Bacc("TRN2", target_bir_lowering=False, debug=False)

    x_d = nc.dram_tensor("x", [NPER, C, H, W], dt.float32, kind="ExternalInput").ap()
    # weights: [co_chunk 2, cin_chunk 2, cin 128, tap 9, co 128]
    w1_d = nc.dram_tensor("w1b", [2, 2, 128, 9, 128], dt.float8e4, kind="ExternalInput").ap()
    w2_d = nc.dram_tensor("w2b", [2, 2, 128, 9, 128], dt.float8e4, kind="ExternalInput").ap()
    # bn holds [s1, t1, s2, t2] x [co_chunk 2] x [128]
    bn_d = nc.dram_tensor("bn", [4, 2, 128], dt.float32, kind="ExternalInput").ap()
    out_d = nc.dram_tensor("out", [NPER, C, H, W], dt.float32, kind="ExternalOutput").ap()

    with tile.TileContext(nc) as tc:
        with (
            tc.tile_pool(name="wp", bufs=1) as wp,
            tc.tile_pool(name="xin", bufs=4) as xinp,
            tc.tile_pool(name="xb", bufs=2) as xbp,
            tc.tile_pool(name="ab", bufs=2) as abp,
            tc.tile_pool(name="ost", bufs=3) as ostp,
            tc.tile_pool(name="tmp", bufs=4) as tmpp,
            tc.tile_pool(name="ps", bufs=7, space="PSUM") as psp,
            nc.sbuf_tensor([128, 2, 128], dt.float8e4) as warm_in,
            nc.psum_tensor([128, 128], dt.float32) as warm_ps,
        ):
            w_sb = []
            for tag in ("w1", "w2"):
                w_sb.append(
                    wp.tile([128, 2, 2, 9, 128], dt.float8e4, tag=tag,
                            name=f"w_{tag}"))
            bn_sb = wp.tile([128, 4, 2], dt.float32, tag="bn")

            # ---- head: queue-balanced loads so the first chunk's data,
            # weights and BN params all land by ~13us.
            # scalar queue: weights/BN configs (their transfers overlap the
            # image-0 piece transfers running on the sync queue).
            nc.scalar.dma_start(
                w_sb[0][:, 0],
                w1_d[0].rearrange("q p k c -> p q k c"))
            nc.scalar.dma_start(bn_sb[:], bn_d.rearrange("f q p -> p f q"))
            nc.scalar.dma_start(
                w_sb[0][:, 1],
                w1_d[1].rearrange("q p k c -> p q k c"))

            # image 0, piecewise, both planes interleaved on the sync queue;
            # Sign (ScalarE) right after each piece lands.  The first chunk's
            # matmuls only need rows 0-9 of both planes.
            pieces0 = ((0, 12), (12, 16), (28, 28))
            xb_cur = xbp.tile([128, PLSZ], dt.float8e4, tag="xb")
            xbv0 = _rview(xb_cur)
            xin_cur = [
                xinp.tile([128, HW], dt.float32, tag="xin", name=f"xin0_{q}")
                for q in range(2)
            ]
            for r0, nr in pieces0:
                for q in range(2):
                    nc.sync.dma_start(
                        xin_cur[q][:, r0 * W:(r0 + nr) * W],
                        x_d[0, q * 128:(q + 1) * 128, r0:r0 + nr].rearrange(
                            "p h w -> p (h w)"),
                    )
                    nc.scalar.activation(
                        xbv0[:, 1 + r0:1 + r0 + nr, q, 1:57],
                        xin_cur[q][:, r0 * W:(r0 + nr) * W].rearrange(
                            "p (h w) -> p h w", w=W),
                        AF.Sign,
                    )
            # w2 on gpsimd behind the xb pad memsets (needed from conv2 on).
            _zero_pads(nc, xb_cur)
            for co in range(2):
                nc.gpsimd.dma_start(
                    w_sb[1][:, co], w2_d[co].rearrange("q p k c -> p q k c"))

            # ---- PE warm-up: small junk matmuls (FD=64) so the PE never
            # idles between the framework preamble and the first real matmul
            # (an idle >3.4us re-throttles the HAM clock gate).  The memset
            # goes on the otherwise-idle DVE so the first warm-up can issue
            # the moment the preamble ends.
            nc.vector.memset(warm_in[:], 0.0)
            for _ in range(N_WARM):
                nc.tensor.matmul(
                    warm_ps[:, 0:64], warm_in[:], warm_in[:, :, 0:64],
                    start=True, stop=True, perf_mode=PM.DoubleRow,
                )

            def prefetch_image(n):
                """DMA image n (q0 on sync, q1 on gpsimd) and binarize into a
                fresh padded xb tile.  Returns (xb_tile, [xin_q0, xin_q1])."""
                xb = xbp.tile([128, PLSZ], dt.float8e4, tag="xb")
                xbv = _rview(xb)
                xin = []
                for q in range(2):
                    xi = xinp.tile([128, HW], dt.float32, tag="xin")
                    xin.append(xi)
                    dma_eng = nc.sync if q == 0 else nc.gpsimd
                    dma_eng.dma_start(
                        xi[:],
                        x_d[n, q * 128:(q + 1) * 128].rearrange("p h w -> p (h w)"),
                    )
                    nc.scalar.activation(
                        xbv[:, 1:57, q, 1:57],
                        xi[:].rearrange("p (h w) -> p h w", w=W),
                        AF.Sign,
                    )
                _zero_pads(nc, xb)
                return xb, xin

            for n in range(NPER):
                xbv = _rview(xb_cur)

                # ---- conv1 -> sign(bn1(.)) into padded intermediate ----
                ab = abp.tile([128, PLSZ], dt.float8e4, tag="ab")
                _zero_pads(nc, ab)
                abv = _rview(ab)
                for co in range(2):
                    for s in range(NCH):
                        ps = psp.tile([128, CHU], dt.float32, tag="ps")
                        for kk in range(9):
                            r0 = RPC * s + kk // 3
                            rhs = xbv[:, r0:r0 + RPC, :, kk % 3:kk % 3 + W].rearrange(
                                "p r k c -> p k r c")
                            nc.tensor.matmul(
                                ps[:],
                                w_sb[0][:, co, :, kk, :],
                                rhs,
                                start=(kk == 0),
                                stop=(kk == 8),
                                perf_mode=PM.DoubleRow,
                            )
                        psv = ps[:].rearrange("p (r c) -> p r c", c=W)
                        nc.scalar.activation(
                            abv[:, 1 + RPC * s:1 + RPC * s + RPC, co, 1:57], psv, AF.Sign,
                            bias=bn_sb[:, 1, co:co + 1], scale=bn_sb[:, 0, co:co + 1],
                        )

                # ---- prefetch + binarize input of image n+1 ----
                # DMAs start now; the Sign ops sit between conv1's and conv2's
                # epilogue activations on the in-order ScalarE queue, done
                # long before conv1 of image n+1 needs them.
                if n + 1 < NPER:
                    xb_nxt, xin_nxt = prefetch_image(n + 1)

                # ---- conv2 -> bn2 + residual + clip ----
                for co in range(2):
                    ost = ostp.tile([128, HW], dt.float32, tag="ost")
                    ostv = ost[:].rearrange("p (h w) -> p h w", w=W)
                    xinv = xin_cur[co][:].rearrange("p (h w) -> p h w", w=W)
                    st_eng = nc.sync if co == 0 else nc.scalar
                    for s in range(NCH):
                        ps = psp.tile([128, CHU], dt.float32, tag="ps")
                        for kk in range(9):
                            r0 = RPC * s + kk // 3
                            rhs = abv[:, r0:r0 + RPC, :, kk % 3:kk % 3 + W].rearrange(
                                "p r k c -> p k r c")
                            nc.tensor.matmul(
                                ps[:],
                                w_sb[1][:, co, :, kk, :],
                                rhs,
                                start=(kk == 0),
                                stop=(kk == 8),
                                perf_mode=PM.DoubleRow,
                            )
                        psv = ps[:].rearrange("p (r c) -> p r c", c=W)
                        tm = tmpp.tile([128, RPC * W], dt.float32, tag="tmp")
                        tmv = tm[:].rearrange("p (r c) -> p r c", c=W)
                        nc.scalar.activation(
                            tmv, psv, AF.Identity,
                            bias=bn_sb[:, 3, co:co + 1], scale=bn_sb[:, 2, co:co + 1],
                        )
                        ov = ostv[:, RPC * s:RPC * s + RPC, :]
                        nc.vector.tensor_tensor(
                            ov, tmv, xinv[:, RPC * s:RPC * s + RPC, :], ALU.add
                        )
                        nc.vector.tensor_scalar(ov, ov, 1.0, -1.0, ALU.min, ALU.max)
                        r0o, r1o = 8 * s, 8 * s + 8
                        st_eng.dma_start(
                            out_d[n, co * 128:(co + 1) * 128, r0o:r1o].rearrange(
                                "p h w -> p (h w)"),
                            ost[:, r0o * W:r1o * W],
                        )

                if n + 1 < NPER:
                    xb_cur, xin_cur = xb_nxt, xin_nxt

    nc.compile()
    return nc


def _get_nc():
    if "nc" not in _CACHE:
        _CACHE["nc"] = _build()
    return _CACHE["nc"]


def _prep_weights(w):
    # [co, cin, kh, kw] ->
    # [co_chunk 2, cin_chunk 2, cin 128, tap 9, co 128], binarized fp8e4
    a = np.sign(w.astype(np.float32))          # [co 256, ci 256, 3, 3]
    a = a.reshape(2, 128, 2, 128, 9)           # [coq, c, q, p, k]
    a = a.transpose(0, 2, 3, 4, 1)             # [coq, q, p, k, c]
    return np.ascontiguousarray(a.astype(ml_dtypes.float8_e4m3))


def _fold_bn(g, b, m, v):
    s = (g.astype(np.float32) / np.sqrt(v.astype(np.float32) + BN_EPS)).astype(np.float32)
    t = (b.astype(np.float32) - m.astype(np.float32) * s).astype(np.float32)
    return s.reshape(2, 128), t.reshape(2, 128)


def _make_in_maps(x, w1, g1, b1, m1, v1, w2, g2, b2, m2, v2):
    w1b = _prep_weights(w1)
    w2b = _prep_weights(w2)
    s1, t1 = _fold_bn(g1, b1, m1, v1)
    s2, t2 = _fold_bn(g2, b2, m2, v2)
    bn = np.ascontiguousarray(np.stack([s1, t1, s2, t2], axis=0))
    x = np.ascontiguousarray(x.astype(np.float32, copy=False))
    return [
        {
            "x": x[c * NPER:(c + 1) * NPER],
            "w1b": w1b, "w2b": w2b, "bn": bn,
        }
        for c in range(N_CORES)
    ]


def kernel(x, w1, g1, b1, m1, v1, w2, g2, b2, m2, v2):
    nc = _get_nc()
    in_maps = _make_in_maps(x, w1, g1, b1, m1, v1, w2, g2, b2, m2, v2)
    res = run_bass_kernel_spmd(nc, in_maps, list(range(N_CORES)))
    out = np.concatenate([res.results[c]["out"] for c in range(N_CORES)], axis=0)
    return out
